# revision 8
# baseline (speedup 1.0000x reference)
"""Trainium2 Bass kernel v2 for CommittorNetBP (pairwise min-image env sum + tiny MLP).

Mathematically equivalent reformulation (see kernel_v1 docstring for the
original derivation); v2 restructures for speed:

 1. wrap(dx)^2 ~= B0 + sum_n Bn cos(2*pi*n*dx/L): pairwise d2 becomes an
    inner product of 96-row trig embeddings (no const row: 3*B0 rides in
    the Exp bias).  The embedding phase matmul runs as TWO bf16 matmuls
    (x/L split hi+lo against exact integer-n stationary rows) — exact to
    ~1e-5 turns at 2 cycles/col instead of fp32's 4.
 2. Pair matmul in fp16 (1 cycle/col): E, Ew stored fp16; measured
    end-to-end rel err ~1.1e-2 vs the 2e-2 gate.
 3. Upper-triangle-only pair blocks (bi <= bj): each computed er block
    feeds BOTH a row-sum (DVE scalar_tensor_tensor accum) and a col-sum
    (PE ones-matmul) — 62.5% of the full-matrix work.
 4. Envelope f(t) ~= w0 + w1 u + w2 u^2 with u = e^{-a t} (alpha2 = 2*alpha1
    constrained fit, a=0.70): ONE Exp pass; the quadratic is a single fused
    DVE scalar_tensor_tensor v' = (u + w1/w2)*u with row-sum accumulated;
    w2 is folded into W1^T on the host, w0 and the diagonal into b1'.
 5. Startup: DMAs issued from sync/scalar/gpsimd queues in parallel,
    PE warmed up with dummy matmuls during the DMA wait, sin ACT table
    preloaded.  Sins grouped before Exps (disjoint ACT table sets).

Sharding: pure data parallel, batch 128 -> 8 cores x 16.
"""

import numpy as np
import ml_dtypes

# ---------------------------------------------------------------- constants
L = 10.0
PI = float(np.pi)
NP = 512
BTOT = 128
NCORES = 8
BLOC = BTOT // NCORES  # 16
NH = 16
K = 6 * NH              # 96 embedding rows (no const row)
NUM_NODES = 256
XW = 96 + BLOC * NP     # xx tile cols: [mt | xa]

B_HARM = [
    8.336507198660753, -10.134305777836879, 2.5283072633082164,
    -1.1207547738471013, 0.6351791173907125, -0.41237594667899846,
    0.28478810229590223, -0.20163605059415754, 0.15059719920404221,
    -0.12490354747428888, 0.11118898587488348, -0.09477489833163562,
    0.06985971056432684, -0.041620415059490684, 0.018837434788739185,
    -0.005869820105041354, 0.0009762178400180537,
]

# envelope fit f(t) ~= CW0 + CW1*e^{-A_ENV t} + CW2*e^{-2 A_ENV t}
A_ENV = 0.70
CW0 = -1.18809612e-06
CW1 = -3.03076726e-02
CW2 = 1.03030886e+00
C_STT = CW1 / CW2                    # v' = (u + C_STT) * u ; inputt = CW2*sum v' + ...
EXP_BIAS = -A_ENV * 3.0 * B_HARM[0]  # folded const row

N_WARM = 16
# bisect knobs
USE_FAST_DMA = False    # sync/scalar-engine DMA queues (else all gpsimd)
USE_WARMUP = False      # PE warmup matmuls
USE_SIN_PRELOAD = False # dummy sin to preload ACT table

f32 = np.float32
bf16 = ml_dtypes.bfloat16


def _host_constants():
    mtb = np.zeros((4, K), f32)   # stationary: integer n + phase row
    bcol = np.zeros((K, 1), f32)
    col = 0
    for c in range(3):
        for n in range(1, NH + 1):
            for phase in (0.25, 0.0):
                mtb[c, col] = float(n)
                mtb[3, col] = phase
                bcol[col, 0] = B_HARM[n]
                col += 1
    eye16 = np.eye(16, dtype=f32)
    return mtb, bcol, eye16


_CACHE = {}


def _build_program():
    import concourse.bacc as bacc
    import concourse.mybir as mybir
    import concourse.tile as tile

    nc = bacc.Bacc("TRN2", target_bir_lowering=False, debug=False,
                   num_devices=NCORES)
    dt = mybir.dt
    AF = mybir.ActivationFunctionType
    ALU = mybir.AluOpType

    xh_d = nc.declare_dram_parameter("xh", (4, XW), dt.bfloat16, isOutput=False)
    xl_d = nc.declare_dram_parameter("xl", (4, XW), dt.bfloat16, isOutput=False)
    cf1_d = nc.declare_dram_parameter("cf1", (K, 17), dt.float32, isOutput=False)
    b1p_d = nc.declare_dram_parameter("b1p", (1, NUM_NODES), dt.float32, isOutput=False)
    w2r_d = nc.declare_dram_parameter("w2r", (16, NUM_NODES), dt.float32, isOutput=False)
    w1t_d = nc.declare_dram_parameter("w1t", (NP, NUM_NODES), dt.bfloat16, isOutput=False)
    y_d = nc.declare_dram_parameter("y", (BLOC, 1), dt.float32, isOutput=True)

    # per-batch pair-block geometry (upper triangle, PSUM-bank aligned):
    #  tile A [128, 1024]: bi0 at [0,512), bi1 at [512,896)
    #  tile B [128, 512]:  bi2 at [0,256), bi3 at [256,384)
    BI_W = [512, 384, 256, 128]
    A_OFF = [0, 512]            # offsets of bi0, bi1 in tile A
    B_OFF = [0, 256]            # offsets of bi2, bi3 in tile B

    with tile.TileContext(nc) as tc:
        with tc.tile_pool(name="const", bufs=1) as cpool:
            # ---------------- input DMAs (parallel queues) ----------------
            xh_s = cpool.tile([4, XW], dt.bfloat16)
            xl_s = cpool.tile([4, XW], dt.bfloat16)
            qs = XW // 4
            e1 = nc.sync if USE_FAST_DMA else nc.gpsimd
            e2 = nc.scalar if USE_FAST_DMA else nc.gpsimd
            cf1_s = cpool.tile([K, 17], dt.float32)
            e2.dma_start(cf1_s[:], cf1_d[:])
            e1.dma_start(xh_s[:, 0:qs], xh_d[:, 0:qs])
            e1.dma_start(xl_s[:, 0:qs], xl_d[:, 0:qs])
            e1.dma_start(xh_s[:, qs:2 * qs], xh_d[:, qs:2 * qs])
            e1.dma_start(xl_s[:, qs:2 * qs], xl_d[:, qs:2 * qs])
            e2.dma_start(xh_s[:, 2 * qs:3 * qs], xh_d[:, 2 * qs:3 * qs])
            e2.dma_start(xl_s[:, 2 * qs:3 * qs], xl_d[:, 2 * qs:3 * qs])
            nc.gpsimd.dma_start(xh_s[:, 3 * qs:XW], xh_d[:, 3 * qs:XW])
            nc.gpsimd.dma_start(xl_s[:, 3 * qs:XW], xl_d[:, 3 * qs:XW])
            b1p_s = cpool.tile([1, NUM_NODES], dt.float32)
            nc.gpsimd.dma_start(b1p_s[:], b1p_d[:])
            w2r_s = cpool.tile([16, NUM_NODES], dt.float32)
            nc.gpsimd.dma_start(w2r_s[:], w2r_d[:])
            w1t_s = cpool.tile([128, 4 * NUM_NODES], dt.bfloat16)
            for c in range(4):
                nc.gpsimd.dma_start(
                    w1t_s[:, c * NUM_NODES:(c + 1) * NUM_NODES],
                    w1t_d[c * 128:(c + 1) * 128, :])

            bcol_ap = cf1_s[0:K, 0:1]
            eye_s = cpool.tile([16, 16], dt.float32)
            nc.vector.tensor_copy(eye_s[:], cf1_s[0:16, 1:17])
            eye_ap = eye_s[0:16, 0:16]
            b1p_ap = b1p_s[0:1, :]
            w2r_ap = w2r_s[0:16, :]

            warm = cpool.tile([1, 128], dt.bfloat16)
            nc.vector.memset(warm[:], 1.0)
            ebias = cpool.tile([128, 1], dt.float32)
            nc.vector.memset(ebias[:], EXP_BIAS)
            wsin_i = cpool.tile([1, 16], dt.float32)
            nc.vector.memset(wsin_i[:], 0.25)
            wsin_o = cpool.tile([1, 16], dt.float32)
            oh_bf = cpool.tile([128, 31], dt.bfloat16)
            nc.vector.memset(oh_bf[:], 0.0)
            nc.vector.memset(oh_bf[:, 15:16], 1.0)
            ones1 = cpool.tile([1, BLOC], dt.bfloat16)
            nc.vector.memset(ones1[:], 1.0)
            b1pb_s = cpool.tile([1, NUM_NODES], dt.bfloat16)
            nc.vector.tensor_copy(b1pb_s[:], b1p_s[:])

            # preload the trig ACT table during the DMA wait
            if USE_SIN_PRELOAD:
                nc.scalar.activation(wsin_o[:], wsin_i[:], AF.Sin,
                                     scale=2.0 * PI)

            E_s = cpool.tile([K, BLOC * NP], dt.float16)
            Ew_s = cpool.tile([K, BLOC * NP], dt.float16)
            racc = [cpool.tile([128, BLOC], dt.float32, name=f"racc{bi}")
                    for bi in range(4)]

            # ---------------- PE warmup during DMA wait ----------------
            if USE_WARMUP:
                with tc.tile_pool(name="wp", bufs=1, space="PSUM") as wp:
                    wt = wp.tile([1, 128], dt.float32)
                    for _ in range(N_WARM):
                        nc.tensor.matmul(wt[:], warm[0:1, 0:1], warm[0:1, :],
                                         start=True, stop=True,
                                         skip_group_check=True)

            # ---------------- phase 1: trig embeddings ----------------
            with (
                tc.tile_pool(name="up", bufs=2, space="PSUM") as upool,
                tc.tile_pool(name="ri", bufs=2) as ripool,
                tc.tile_pool(name="vf", bufs=2) as vfpool,
            ):
                for b in range(BLOC):
                    c0 = 96 + b * NP
                    u = upool.tile([K, NP], dt.float32, tag="u")
                    nc.tensor.matmul(u[:], xh_s[:, 0:96],
                                     xh_s[:, c0:c0 + NP],
                                     start=True, stop=False)
                    nc.tensor.matmul(u[:], xl_s[:, 0:96],
                                     xl_s[:, c0:c0 + NP],
                                     start=False, stop=True)
                    ri = ripool.tile([K, NP], dt.int32, tag="ri")
                    nc.vector.tensor_copy(ri[:], u[:])
                    vfrac = vfpool.tile([K, NP], dt.float32, tag="vf")
                    nc.vector.tensor_tensor(vfrac[:], u[:], ri[:],
                                            ALU.subtract)
                    nc.scalar.activation(E_s[:, b * NP:(b + 1) * NP], vfrac[:],
                                         AF.Sin, scale=2.0 * PI)
                    nc.gpsimd.tensor_scalar(Ew_s[:, b * NP:(b + 1) * NP],
                                            E_s[:, b * NP:(b + 1) * NP],
                                            bcol_ap, None, ALU.mult)

            # ---------------- phase 2: pair blocks ----------------
            with tc.tile_pool(name="cs", bufs=1, space="PSUM") as cspool:
              csum = cspool.tile([BLOC, 384], dt.float32)
              with (
                tc.tile_pool(name="tA", bufs=2, space="PSUM") as tApool,
                tc.tile_pool(name="tB", bufs=1, space="PSUM") as tBpool,
                tc.tile_pool(name="ua", bufs=2) as uapool,
                tc.tile_pool(name="ub", bufs=2) as ubpool,
                tc.tile_pool(name="va", bufs=3) as vapool,
                tc.tile_pool(name="vb", bufs=3) as vbpool,
              ):
                vA_l, vB_l = [None] * BLOC, [None] * BLOC

                def emit_pair(b):
                    boff = b * NP
                    tA = tApool.tile([128, 1024], dt.float32, tag="tA")
                    tB = tBpool.tile([128, 512], dt.float32, tag="tB")
                    for bi in range(4):
                        t_ap = (tA[:, A_OFF[bi]:A_OFF[bi] + BI_W[bi]] if bi < 2
                                else tB[:, B_OFF[bi - 2]:B_OFF[bi - 2] + BI_W[bi]])
                        nc.tensor.matmul(
                            t_ap,
                            Ew_s[:, boff + bi * 128: boff + bi * 128 + 128],
                            E_s[:, boff + bi * 128: boff + NP],
                            start=True, stop=True)
                    uA = uapool.tile([128, 896], dt.bfloat16, tag="uA")
                    nc.scalar.activation(uA[:], tA[:, 0:896], AF.Exp,
                                         scale=-A_ENV, bias=ebias[:, 0:1])
                    uB = ubpool.tile([128, 384], dt.bfloat16, tag="uB")
                    nc.scalar.activation(uB[:], tB[:, 0:384], AF.Exp,
                                         scale=-A_ENV, bias=ebias[:, 0:1])
                    vA = vapool.tile([128, 896], dt.bfloat16, tag="vA")
                    vB = vbpool.tile([128, 384], dt.bfloat16, tag="vB")
                    for bi in range(4):
                        if bi < 2:
                            u_ap = uA[:, A_OFF[bi]:A_OFF[bi] + BI_W[bi]]
                            v_ap = vA[:, A_OFF[bi]:A_OFF[bi] + BI_W[bi]]
                        else:
                            u_ap = uB[:, B_OFF[bi - 2]:B_OFF[bi - 2] + BI_W[bi]]
                            v_ap = vB[:, B_OFF[bi - 2]:B_OFF[bi - 2] + BI_W[bi]]
                        nc.vector.scalar_tensor_tensor(
                            v_ap, u_ap, C_STT, u_ap, ALU.add, ALU.mult,
                            accum_out=racc[bi][:, b:b + 1])
                    vA_l[b], vB_l[b] = vA, vB

                def emit_colsum(b):
                    vA, vB = vA_l[b], vB_l[b]
                    oh = oh_bf[:, 15 - b:31 - b]
                    # strictly-upper slices -> inputt[j] partial sums
                    # csum cols map j in [128, 512) -> [0, 384)
                    nc.tensor.matmul(csum[:, 0:384], oh, vA[:, 128:512],
                                     start=(b == 0), stop=False,
                                     skip_group_check=True)
                    nc.tensor.matmul(csum[:, 128:384], oh, vA[:, 640:896],
                                     start=False, stop=False,
                                     skip_group_check=True)
                    nc.tensor.matmul(csum[:, 256:384], oh, vB[:, 128:256],
                                     start=False, stop=(b == BLOC - 1),
                                     skip_group_check=True)
                    vA_l[b] = vB_l[b] = None

                for b in range(BLOC):
                    emit_pair(b)
                    if b >= 2:
                        emit_colsum(b - 2)
                emit_colsum(BLOC - 2)
                emit_colsum(BLOC - 1)

              # ---------------- phase 3: MLP tail ----------------
              if True:
                with (
                    tc.tile_pool(name="trp", bufs=3, space="PSUM") as trpool,
                    tc.tile_pool(name="hp", bufs=1, space="PSUM") as hpool,
                    tc.tile_pool(name="tail", bufs=1) as tail,
                ):
                    scopy = tail.tile([BLOC, 384], dt.float32)
                    nc.scalar.activation(scopy[:], csum[:], AF.Copy)
                    it_l = []
                    it0 = tail.tile([128, BLOC], dt.bfloat16, name="it0")
                    nc.vector.tensor_scalar(it0[:], racc[0][:], 1.0, None,
                                            ALU.mult)
                    it_l.append(it0)
                    for c in range(1, 4):
                        tp = trpool.tile([128, BLOC], dt.float32, tag="tp")
                        nc.tensor.transpose(
                            tp[:], scopy[:, (c - 1) * 128:c * 128], eye_ap)
                        itc = tail.tile([128, BLOC], dt.bfloat16, name=f"it{c}")
                        nc.vector.tensor_tensor(itc[:], racc[c][:], tp[:],
                                                ALU.add)
                        it_l.append(itc)
                    h = hpool.tile([BLOC, NUM_NODES], dt.float32)
                    for c in range(4):
                        nc.tensor.matmul(
                            h[:], it_l[c][:],
                            w1t_s[:, c * NUM_NODES:(c + 1) * NUM_NODES],
                            start=(c == 0), stop=False, skip_group_check=True)
                    nc.tensor.matmul(h[:], ones1[:], b1pb_s[0:1, :],
                                     start=False, stop=True,
                                     skip_group_check=True)
                    hr = tail.tile([BLOC, NUM_NODES], dt.float32)
                    nc.scalar.activation(hr[:], h[:], AF.Relu)
                    hw = tail.tile([BLOC, NUM_NODES], dt.float32)
                    nc.vector.tensor_tensor(hw[:], hr[:], w2r_ap, ALU.mult)
                    z = tail.tile([BLOC, 1], dt.float32)
                    nc.vector.reduce_sum(z[:], hw[:], axis=mybir.AxisListType.X)
                    th = tail.tile([BLOC, 1], dt.float32)
                    nc.scalar.activation(th[:], z[:], AF.Tanh, scale=0.5)
                    ys = tail.tile([BLOC, 1], dt.float32)
                    nc.vector.tensor_scalar(ys[:], th[:], 0.5, 0.5,
                                            ALU.mult, ALU.add)
                    nc.gpsimd.dma_start(y_d[:], ys[:])

    nc.finalize()
    return nc


def _get_program():
    if "nc" not in _CACHE:
        _CACHE["nc"] = _build_program()
    return _CACHE["nc"]


def _make_in_maps(x, W1, b1, W2):
    mtb, bcol, eye16 = _host_constants()
    W1 = np.asarray(W1, f32)
    w1t = np.ascontiguousarray(W1.T * f32(CW2)).astype(bf16)
    b1p = (np.asarray(b1, f32)
           + (NP * f32(CW0) - 1.0) * W1.sum(axis=1)).reshape(1, NUM_NODES).astype(f32)
    cf1 = np.zeros((K, 17), f32)
    cf1[:, 0:1] = bcol
    cf1[0:16, 1:17] = eye16
    w2r = np.broadcast_to(np.asarray(W2, f32).reshape(1, NUM_NODES),
                          (16, NUM_NODES)).copy()
    x = np.asarray(x, f32)
    in_maps = []
    for c in range(NCORES):
        xs = x[c * BLOC:(c + 1) * BLOC]                         # [16,512,3]
        xT = np.transpose(xs, (2, 0, 1)).reshape(3, BLOC * NP)  # [3,16*512]
        x10 = (xT / f32(L)).astype(f32)
        xh = x10.astype(bf16)
        xl = (x10 - xh.astype(f32)).astype(bf16)
        xha = np.zeros((4, XW), bf16)
        xla = np.zeros((4, XW), bf16)
        xha[0:4, 0:96] = mtb.astype(bf16)
        xla[0:4, 0:96] = mtb.astype(bf16)
        xha[0:3, 96:] = xh
        xha[3, 96:] = bf16(1.0)
        xla[0:3, 96:] = xl
        xla[3, 96:] = bf16(0.0)
        in_maps.append({"xh": xha, "xl": xla, "cf1": cf1, "b1p": b1p,
                        "w2r": w2r, "w1t": w1t})
    return in_maps


def kernel(x, W1, b1, W2, _trace=False, _trace_kwargs=None):
    from concourse.bass_utils import run_bass_kernel_spmd

    nc = _get_program()
    in_maps = _make_in_maps(x, W1, b1, W2)
    res = run_bass_kernel_spmd(nc, in_maps, list(range(NCORES)),
                               trace=_trace, **(_trace_kwargs or {}))
    out = np.concatenate([res.results[c]["y"] for c in range(NCORES)], axis=0)
    if _trace:
        _CACHE["last_result"] = res
    return out.astype(f32)
